# revision 21
# baseline (speedup 1.0000x reference)
"""2-layer GAT on 8 Trainium2 NeuronCores (Bass/Tile) — V3.

Strategy (dst-per-partition graph parallelism):
  Host: sort nodes by degree; 392 bins of 128 nodes; partition p of a bin owns
  dst node bin[p] and ALL its incoming edges live in partition p's chunk row.
  Per-dst alpha_dst broadcasts along the free dim (no per-edge dst gather);
  the segment softmax denominator and the aggregation are free-dim tree
  reductions on DVE (no selection-matrix matmuls).

  Performance structure:
  - Pad slots gather a host-written POISON row (alpha_src = -1000) so their
    exp() underflows to exactly 0: no mask multiplies anywhere.
  - Feature columns stored in (k, head) order so the per-head weight
    broadcast is not innermost -> DVE 2x/4x packed modes stay enabled.
  - Hot elementwise ops use scalar_tensor_tensor (4x-capable opcode).
  - All index/mask tables prefetched in one DMA per iteration.
  - NEFF-A: one big load, 49 bf16 matmuls, one big store.
  - NEFF-B: g rows 512B fp16 ([h 128 | as 8 | ad 8 | pad]); int16 indices
    address two overlapping views (rows 0..32767 / 17408..50175); node 0's row
    is swapped with row 50000 so row 0 can be poison. Mid-range srcs
    (17408..32767) flex between the two gather calls to balance sections.
  - NEFF-C: pair-packed table (2 nodes per 256B unit); parity select on-chip;
    single table view, poison unit 25000.

  8 cores x 49 bins; bins rank-grouped in 8s (one per core) with shared
  shapes so all cores run one NEFF (SPMD).
"""

import sys

sys.path.insert(0, "/opt/trn_rl_repo")

import numpy as np

import concourse.bacc as bacc
import concourse.bass as bass
import concourse.mybir as mybir
import concourse.tile as tile

# ---------------- problem constants (hardcoded per task contract) -------------
N = 50000
F_IN = 128
HID = 16
HEADS = 8
CLASSES = 16
NEG = 0.2

N_CORES = 8
P = 128
BLOCKS_PER_CORE = 49
NBINS = N_CORES * BLOCKS_PER_CORE          # 392
N_PAD = NBINS * P                          # 50176
NODES_PER_CORE = N_PAD // N_CORES          # 6272

LOA = 32768                                # table view A = rows [0, 32768)
MIDBASE = 17408                            # table view B = rows [17408, 50176)
POISON_LO = 0                              # row 0 (node 0 relocated to 50000)
NODE0_ROW = 50000
POISON_HI = N_PAD - 1                      # row 50175 (pad node)

GROW = 256                                 # g row elems (fp16) = 512B
G2SUB = 64                                 # g2 sub-row elems (fp16) = 128B
G2UNIT = 128                               # g2 unit elems (2 nodes) = 256B
NUNIT = N_PAD // 2                         # 25088 pair units
POISON_UNIT = 25000                        # unit of pad nodes 50000/50001
M_SHIFT = 4.0                              # softmax-invariant exp shift
GMAX = 1024                                # max indices per dma_gather call
BCAP = 32                                  # B pair-fused chunk capacity
CCAP = 24                                  # C quad-fused chunk capacity
NQ = 4                                     # SWDGE queues

F16 = mybir.dt.float16
F32 = mybir.dt.float32
I16 = mybir.dt.int16

# feature permutation: stored col k*8+h = logical head-major col h*16+k
PERM_KH = np.arange(128).reshape(HEADS, HID).T.reshape(-1)

_cache = {}
_last_cfg = None
_last_inputs = None


# ---------------------------- host preprocessing -----------------------------

def _wrap16(a):
    """Index array [n] -> dma_gather SBUF layout [128, n/16]."""
    n = a.shape[0]
    assert n % 16 == 0
    w = a.reshape(n // 16, 16).T.astype(np.int16)
    return np.tile(w, (8, 1))


def _group_shapes_b(lmin, m, hmin, bins):
    """Per 8-bin rank group: single (L, H) so every core runs one program."""
    Ls, Hs = [], []
    for g in range(BLOCKS_PER_CORE):
        nodes = bins[g * 8:(g + 1) * 8].reshape(-1)
        lo, mi, hi = lmin[nodes], m[nodes], hmin[nodes]
        best = None
        for L in range(int(lo.max()), int((lo + mi).max()) + 1):
            H = int((hi + np.maximum(0, mi - (L - lo))).max())
            if best is None or L + H < best[0]:
                best = (L + H, L, H)
        Ls.append(best[1])
        Hs.append(best[2])
    return Ls, Hs


def host_prep(edge_index):
    src = np.concatenate([edge_index[0].astype(np.int64),
                          np.arange(N, dtype=np.int64)])
    dst = np.concatenate([edge_index[1].astype(np.int64),
                          np.arange(N, dtype=np.int64)])
    # B-table row of each node (node 0 relocated so row 0 can be poison)
    row_of = np.arange(N_PAD, dtype=np.int64)
    row_of[0] = NODE0_ROW
    srow = row_of[src]

    order = np.argsort(dst, kind="stable")
    srow_s = srow[order]
    src_s = src[order]
    deg = np.bincount(dst, minlength=N_PAD)
    estart = np.zeros(N_PAD + 1, np.int64)
    np.cumsum(deg, out=estart[1:])

    lmin = np.bincount(dst[srow < MIDBASE], minlength=N_PAD)
    hmin = np.bincount(dst[srow >= LOA], minlength=N_PAD)
    m = deg - lmin - hmin

    # ---- B binning: sort by (deg, lmin); bins of 128; groups of 8 bins ----
    nodesB = np.lexsort((lmin, deg))
    binsB = nodesB.reshape(NBINS, P)
    LsB, HsB = _group_shapes_b(lmin, m, hmin, binsB)

    # pair bins (fused elementwise stages) while uniform D <= BCAP
    groupsB = []
    j = 0
    while j < BLOCKS_PER_CORE:
        if j + 1 < BLOCKS_PER_CORE:
            D2 = max(LsB[j] + HsB[j], LsB[j + 1] + HsB[j + 1]) + 2
            Dd = abs(LsB[j] + HsB[j] - LsB[j + 1] - HsB[j + 1])
            if D2 <= BCAP and Dd <= 2:
                groupsB.append((j, 2, D2))
                HsB[j] = D2 - 2 - LsB[j]
                HsB[j + 1] = D2 - 2 - LsB[j + 1]
                j += 2
                continue
        groupsB.append((j, 1, LsB[j] + HsB[j] + 2))
        j += 1

    # X chunk layout: 0 = dstA | [1, L+1) lo edges | [L+1, L+1+H) hi edges |
    # D-1 = dstB.  D = L+H+2.  Edge range = [1, D-1).
    siB_parts, mB_parts = [], []
    for k in range(N_CORES):
        si_list, adm_list = [], []
        for j in range(BLOCKS_PER_CORE):
            L, H = LsB[j], HsB[j]
            D = L + H + 2
            nodes = binsB[j * 8 + k]
            Alo = np.full((L + 1, P), POISON_LO, np.int64)
            Ahi = np.full((H + 1, P), POISON_HI - MIDBASE, np.int64)
            adm = np.zeros((P, 1), np.float32)
            for p in range(P):
                nd = nodes[p]
                es = srow_s[estart[nd]:estart[nd + 1]]
                elo = es[es < MIDBASE]
                emid = es[(es >= MIDBASE) & (es < LOA)]
                ehi = es[es >= LOA]
                x = min(len(emid), L - len(elo))
                lo_list = np.concatenate([elo, emid[:x]])
                hi_list = np.concatenate([emid[x:], ehi])
                assert len(lo_list) <= L and len(hi_list) <= H
                Alo[1:1 + len(lo_list), p] = lo_list
                Ahi[:len(hi_list), p] = hi_list - MIDBASE
                ndr = row_of[nd]
                if ndr < LOA:
                    Alo[0, p] = ndr
                    adm[p] = 1.0
                else:
                    Ahi[H, p] = ndr - MIDBASE
            si_list.append(_wrap16(Alo.reshape(-1)))
            si_list.append(_wrap16(Ahi.reshape(-1)))
            adm_list.append(adm)
        siB_parts.append(np.concatenate(si_list, axis=1))
        mB_parts.append(np.concatenate(adm_list, axis=1))
    siB = np.stack(siB_parts)                       # [8, 128, WB]
    mB = np.stack(mB_parts).astype(np.float32)      # [8, 128, 49]

    # ---- C binning: sort by deg; pair-unit table; dst chunk first ----
    nodesC = np.argsort(deg, kind="stable")
    binsC = nodesC.reshape(NBINS, P)
    DsC = []
    for g in range(BLOCKS_PER_CORE):
        nodes = binsC[g * 8:(g + 1) * 8].reshape(-1)
        DsC.append(int(deg[nodes].max()))

    # quad/duo/solo groups with uniform D = De+1
    groupsC = []
    j = 0
    while j < BLOCKS_PER_CORE:
        placed = False
        for g in (4, 2, 1):
            if j + g <= BLOCKS_PER_CORE:
                Dg = max(DsC[j + b] for b in range(g)) + 1
                cap = CCAP if g == 4 else (2 * CCAP if g == 2 else 4 * CCAP)
                if Dg <= cap:
                    groupsC.append((j, g, Dg))
                    for b in range(g):
                        DsC[j + b] = Dg - 1
                    j += g
                    placed = True
                    break
        assert placed, "bin exceeds 4*CCAP"

    siC_parts, mC_parts = [], []
    for k in range(N_CORES):
        si_list, m_list = [], []
        for j in range(BLOCKS_PER_CORE):
            De = DsC[j]
            D = De + 1                               # dst chunk + De edge chunks
            nodes = binsC[j * 8 + k]
            A = np.full((D, P), POISON_UNIT, np.int64)
            par = np.zeros((P, D), np.float32)
            for p in range(P):
                nd = nodes[p]
                es = src_s[estart[nd]:estart[nd + 1]]
                A[0, p] = nd >> 1
                par[p, 0] = float(nd & 1)
                A[1:1 + len(es), p] = es >> 1
                par[p, 1:1 + len(es)] = (es & 1).astype(np.float32)
            si_list.append(_wrap16(A.reshape(-1)))
            m_list.append(par)
        siC_parts.append(np.concatenate(si_list, axis=1))
        mC_parts.append(np.concatenate(m_list, axis=1))
    siC = np.stack(siC_parts)
    mC = np.stack(mC_parts).astype(np.float32)

    return dict(LsB=LsB, HsB=HsB, DsC=DsC, binsB=binsB, binsC=binsC,
                groupsB=groupsB, groupsC=groupsC,
                siB=siB, mB=mB, siC=siC, mC=mC)


# ------------------------------- NEFF builders -------------------------------

def build_neff_a(reps=1):
    nc = bacc.Bacc()
    BF16 = mybir.dt.bfloat16
    xT = nc.dram_tensor("xT", [P, NODES_PER_CORE], BF16, kind="ExternalInput")
    w1e = nc.dram_tensor("w1e", [P, 144], BF16, kind="ExternalInput")
    g_out = nc.dram_tensor("g_out", [P, BLOCKS_PER_CORE, 144], F16,
                           kind="ExternalOutput")
    ntiles = NODES_PER_CORE // P
    with tile.TileContext(nc) as tc:
        with tc.tile_pool(name="sbuf", bufs=2) as pool, \
             tc.tile_pool(name="psum", bufs=4, space="PSUM") as pp:
            w1t = pool.tile([P, 144], BF16)
            nc.sync.dma_start(w1t[:], w1e[:])

            def body():
                xt = pool.tile([P, NODES_PER_CORE], BF16, tag="xt", name="xt")
                nc.sync.dma_start(xt[:], xT[:, :])
                gt = pool.tile([P, ntiles, 144], F16, tag="gt", name="gt")
                for t in range(ntiles):
                    ps = pp.tile([P, 144], F32, tag="ps", space="PSUM", name="ps")
                    nc.tensor.matmul(out=ps[:], lhsT=xt[:, t * P:(t + 1) * P],
                                     rhs=w1t[:], start=True, stop=True)
                    nc.vector.tensor_copy(out=gt[:, t, :], in_=ps[:])
                nc.sync.dma_start(g_out[:, :, :], gt[:])

            for _ in range(reps):
                body()
    nc.finalize()
    return nc


def _gather_calls(nc, out_tile, c0, nchunks, table_ap, si_tile, w0, row, qrr):
    """dma_gather calls (split at GMAX) covering nchunks chunks of out_tile
    starting at chunk c0, indices from si_tile columns w0..."""
    n = nchunks * P
    done = 0
    while done < n:
        cnt = min(GMAX, n - done)
        nc.gpsimd.dma_gather(
            out_ap=out_tile[:, c0 + done // P:c0 + (done + cnt) // P, :],
            in_ap=table_ap,
            idxs_ap=si_tile[:, w0 + done // 16:w0 + (done + cnt) // 16],
            num_idxs=cnt, num_idxs_reg=cnt, elem_size=row,
            queue_num=qrr())
        done += cnt


def _stt(nc, out, in0, scalar, in1, op0, op1):
    nc.vector.scalar_tensor_tensor(out=out, in0=in0, scalar=scalar, in1=in1,
                                   op0=op0, op1=op1)


def _tree_reduce(nc, pool, wx, nmax, nsl, width, tag):
    """Sum wx[:, 0:nsl, 0:width] over the slot axis -> [P, width] f32.
    scalar_tensor_tensor adds (4x DVE mode) on contiguous halves."""
    add = mybir.AluOpType.add
    cur = wx
    n = nsl
    buf = pool.tile([P, (nmax + 1) // 2, width], F16, tag=tag + "_pp")
    while n > 2:
        half = n // 2
        _stt(nc, buf[:, 0:half, :], cur[:, 0:half, :], 0.0,
             cur[:, half:2 * half, :], add, add)
        if n % 2:
            nc.scalar.copy(buf[:, half, :], cur[:, n - 1, :])
        cur, buf = buf, cur
        n = (n + 1) // 2
    out = pool.tile([P, width], F32, tag=tag + "_out")
    if n == 2:
        _stt(nc, out[:], cur[:, 0, :], 0.0, cur[:, 1, :], add, add)
    else:
        nc.vector.tensor_copy(out=out[:], in_=cur[:, 0, :])
    return out


def _gv3(tile, gb, st, lo, hi, e0, e1):
    """Group view of flat [P, CH, W] tile: [P, gb, hi-lo, e1-e0] with
    per-group chunk stride st."""
    return tile[:, 0:gb * st, :].rearrange("p (g c) w -> p g c w", g=gb)[
        :, :, lo:hi, e0:e1]


def _gv2(tile, gb, st, lo, hi):
    return tile[:, 0:gb * st].rearrange("p (g f) -> p g f", g=gb)[:, :, lo:hi]


def _tree_reduce2(nc, pool, wxf, gb, st, nsl, width, tag, ncap):
    """Sum group-view wxf[:, g, 0:nsl, 0:width] (flat tile, stride st) over
    the slot axis -> [P, gb, width] f32 (flat out tile)."""
    add = mybir.AluOpType.add
    curf, curst = wxf, st
    n = nsl
    buff = pool.tile([P, ncap, width], F16, tag=tag + "_pp")
    bufst = ncap // gb
    while n > 2:
        half = n // 2
        nc.vector.tensor_tensor(
            out=_gv3(buff, gb, bufst, 0, half, 0, width),
            in0=_gv3(curf, gb, curst, 0, half, 0, width),
            in1=_gv3(curf, gb, curst, half, 2 * half, 0, width), op=add)
        if n % 2:
            nc.scalar.copy(_gv3(buff, gb, bufst, half, half + 1, 0, width),
                           _gv3(curf, gb, curst, n - 1, n, 0, width))
        curf, curst, buff, bufst = buff, bufst, curf, curst
        n = (n + 1) // 2
    out = pool.tile([P, gb, width], F32, tag=tag + "_out")
    if n == 2:
        nc.vector.tensor_tensor(out=out[:, 0:gb],
                                in0=_gv3(curf, gb, curst, 0, 1, 0, width)[:, :, 0, :],
                                in1=_gv3(curf, gb, curst, 1, 2, 0, width)[:, :, 0, :],
                                op=add)
    else:
        nc.vector.tensor_copy(out=out[:, 0:gb],
                              in_=_gv3(curf, gb, curst, 0, 1, 0, width)[:, :, 0, :])
    return out


def build_neff_b(cfg, reps=1):
    nc = bacc.Bacc(num_swdge_queues=NQ)
    LsB, HsB = cfg["LsB"], cfg["HsB"]
    groupsB = cfg["groupsB"]
    WB = cfg["siB"].shape[2]
    add = mybir.AluOpType.add
    mx = mybir.AluOpType.max
    mult = mybir.AluOpType.mult
    sub = mybir.AluOpType.subtract
    CH = 2 * BCAP

    g_d = nc.dram_tensor("g", [N_PAD, GROW], F16, kind="ExternalInput")
    si_d = nc.dram_tensor("si", [P, WB], I16, kind="ExternalInput")
    m_d = nc.dram_tensor("m", [P, BLOCKS_PER_CORE], F32, kind="ExternalInput")
    w2e_d = nc.dram_tensor("w2e", [P, 18], F16, kind="ExternalInput")
    w2cs_d = nc.dram_tensor("w2cs", [1, 18], F16, kind="ExternalInput")
    ones_d = nc.dram_tensor("ones1", [1, P], F16, kind="ExternalInput")
    b1_d = nc.dram_tensor("b1r", [P, P], F16, kind="ExternalInput")
    ident_d = nc.dram_tensor("ident", [P, P], F16, kind="ExternalInput")
    g2_out = nc.dram_tensor("g2_out", [BLOCKS_PER_CORE, P, 18], F16,
                            kind="ExternalOutput")
    gA = g_d[0:LOA, :]
    gB = g_d[MIDBASE:N_PAD, :]

    qctr = [0]

    def qrr():
        qctr[0] = (qctr[0] + 1) % NQ
        return qctr[0]

    with tile.TileContext(nc) as tc:
        with tc.tile_pool(name="sbuf", bufs=2) as pool, \
             tc.tile_pool(name="persist", bufs=1) as pers, \
             tc.tile_pool(name="psum", bufs=2, space="PSUM") as pp:
            mshift = pers.tile([P, 1], F32)
            nc.gpsimd.memset(mshift[:], -M_SHIFT)
            w2t = pers.tile([P, 18], F16)
            nc.sync.dma_start(w2t[:], w2e_d[:])
            w2cs = pers.tile([1, 18], F16)
            nc.sync.dma_start(w2cs[:], w2cs_d[:])
            ones1 = pers.tile([1, P], F16)
            nc.sync.dma_start(ones1[:], ones_d[:])
            b1t = pers.tile([P, P], F16)
            nc.sync.dma_start(b1t[:], b1_d[:])
            identt = pers.tile([P, P], F16)
            nc.sync.dma_start(identt[:], ident_d[:])

            def body():
                si = pers.tile([P, WB], I16, tag="si_all")
                nc.sync.dma_start(si[:], si_d[:])
                mt = pers.tile([P, BLOCKS_PER_CORE], F32, tag="m_all")
                nc.sync.dma_start(mt[:], m_d[:])
                woffs = []
                w = 0
                for j in range(BLOCKS_PER_CORE):
                    woffs.append(w)
                    w += (LsB[j] + HsB[j] + 2) * 8
                for (j0, gb, D) in groupsB:
                    ne = D - 2
                    X = pool.tile([P, CH, GROW], F16, tag="X")
                    for b in range(gb):
                        j = j0 + b
                        L, H = LsB[j], HsB[j]
                        Xb = X[:, b * D:(b + 1) * D, :]
                        _gather_calls(nc, Xb, 0, L + 1, gA, si, woffs[j],
                                      GROW, qrr)
                        _gather_calls(nc, Xb, L + 1, H + 1, gB, si,
                                      woffs[j] + (L + 1) * 8, GROW, qrr)

                    XA = _gv3(X, gb, D, 0, 1, 128, 144)[:, :, 0, :]
                    XBv = _gv3(X, gb, D, D - 1, D, 128, 144)[:, :, 0, :]
                    adm2 = mt[:, j0:j0 + gb]
                    rdd = pool.tile([P, 2, 16], F16, tag="rdd")
                    nc.vector.tensor_tensor(out=rdd[:, 0:gb], in0=XA,
                                            in1=XBv, op=sub)
                    rdm = pool.tile([P, 2, 16], F16, tag="rdm")
                    nc.vector.tensor_tensor(
                        out=rdm[:, 0:gb], in0=rdd[:, 0:gb],
                        in1=adm2[:, :, None].to_broadcast([P, gb, 16]),
                        op=mult)
                    rd = pool.tile([P, 2, 16], F16, tag="rd")
                    nc.vector.tensor_tensor(out=rd[:, 0:gb], in0=rdm[:, 0:gb],
                                            in1=XBv, op=add)

                    Xas = _gv3(X, gb, D, 1, D - 1, 128, 136)
                    Xh = _gv3(X, gb, D, 1, D - 1, 0, 128)
                    t_t = pool.tile([P, CH, HEADS], F16, tag="t")
                    tv = _gv3(t_t, gb, ne, 0, ne, 0, HEADS)
                    nc.vector.tensor_tensor(
                        out=tv, in0=Xas,
                        in1=rd[:, 0:gb, None, 8:16].to_broadcast([P, gb, ne, HEADS]),
                        op=add)
                    e1 = pool.tile([P, CH, HEADS], F16, tag="e1")
                    e1v = _gv3(e1, gb, ne, 0, ne, 0, HEADS)
                    nc.scalar.activation(e1v, tv,
                                         mybir.ActivationFunctionType.Exp,
                                         bias=mshift[:])
                    e2 = pool.tile([P, CH, HEADS], F16, tag="e2")
                    e2v = _gv3(e2, gb, ne, 0, ne, 0, HEADS)
                    nc.scalar.activation(e2v, tv,
                                         mybir.ActivationFunctionType.Exp,
                                         bias=mshift[:], scale=NEG)
                    we = pool.tile([P, CH, HEADS], F16, tag="we")
                    wev = _gv3(we, gb, ne, 0, ne, 0, HEADS)
                    nc.vector.tensor_tensor(out=wev, in0=e1v, in1=e2v, op=mx)

                    wx = pool.tile([P, CH, 136], F16, tag="wx")
                    wxh = _gv3(wx, gb, ne, 0, ne, 0, 128)
                    nc.vector.tensor_tensor(
                        out=wxh.rearrange("p g c (kk hh) -> p g c kk hh", hh=HEADS),
                        in0=Xh.rearrange("p g c (kk hh) -> p g c kk hh", hh=HEADS),
                        in1=_gv3(we, gb, ne, 0, ne, 0, HEADS)[:, :, :, None, :]
                        .to_broadcast([P, gb, ne, HID, HEADS]),
                        op=mult)
                    nc.scalar.copy(_gv3(wx, gb, ne, 0, ne, 128, 136), wev)

                    acc = _tree_reduce2(nc, pool, wx, gb, ne, ne, 136, "trB",
                                        CH // 2 + 2)

                    recip = pool.tile([P, 2, HEADS], F32, tag="recip")
                    nc.vector.reciprocal(recip[:, 0:gb], acc[:, 0:gb, 128:136])
                    o1 = pool.tile([P, 2, P], F16, tag="o1")
                    nc.vector.tensor_tensor(
                        out=o1[:, 0:gb].rearrange("p g (kk hh) -> p g kk hh", hh=HEADS),
                        in0=acc[:, 0:gb, 0:128].rearrange("p g (kk hh) -> p g kk hh", hh=HEADS),
                        in1=recip[:, 0:gb, None, :].to_broadcast([P, gb, HID, HEADS]),
                        op=mult)
                    nc.vector.tensor_tensor(
                        out=o1[:, 0:gb], in0=o1[:, 0:gb],
                        in1=b1t[:, None, :].to_broadcast([P, gb, P]), op=add)
                    vmin = pool.tile([P, 2, P], F16, tag="vmin")
                    nc.vector.tensor_scalar(out=vmin[:, 0:gb], in0=o1[:, 0:gb],
                                            scalar1=0.0, scalar2=None,
                                            op0=mybir.AluOpType.min)
                    ev = pool.tile([P, 2, P], F16, tag="ev")
                    nc.scalar.activation(ev[:, 0:gb], vmin[:, 0:gb],
                                         mybir.ActivationFunctionType.Exp)
                    elu = pool.tile([P, 2, P], F16, tag="elu")
                    nc.vector.tensor_scalar(out=elu[:, 0:gb], in0=o1[:, 0:gb],
                                            scalar1=0.0, scalar2=None,
                                            op0=mybir.AluOpType.max)
                    nc.vector.tensor_tensor(out=elu[:, 0:gb], in0=elu[:, 0:gb],
                                            in1=ev[:, 0:gb], op=add)
                    for b in range(gb):
                        eTp = pp.tile([P, P], F16, tag="eTp", space="PSUM")
                        nc.tensor.transpose(out=eTp[:], in_=elu[:, b, :],
                                            identity=identt[:])
                        eT = pool.tile([P, P], F16, tag="eT")
                        nc.scalar.copy(eT[:], eTp[:])
                        g2p = pp.tile([P, 18], F32, tag="g2p", space="PSUM")
                        nc.tensor.matmul(out=g2p[:], lhsT=eT[:], rhs=w2t[:],
                                         start=True, stop=False)
                        nc.tensor.matmul(out=g2p[:], lhsT=ones1[:], rhs=w2cs[:],
                                         start=False, stop=True)
                        g2t = pool.tile([P, 18], F16, tag="g2t")
                        nc.scalar.copy(g2t[:], g2p[:])
                        nc.sync.dma_start(g2_out[j0 + b], g2t[:])

            for _ in range(reps):
                body()
    nc.finalize()
    return nc


def build_neff_c(cfg, reps=1):
    nc = bacc.Bacc(num_swdge_queues=NQ)
    DsC = cfg["DsC"]
    groupsC = cfg["groupsC"]
    WC = cfg["siC"].shape[2]
    WM = cfg["mC"].shape[2]
    add = mybir.AluOpType.add
    mx = mybir.AluOpType.max
    mult = mybir.AluOpType.mult
    sub = mybir.AluOpType.subtract
    CH = 4 * CCAP

    g4_d = nc.dram_tensor("g4", [NUNIT, G2UNIT], F16, kind="ExternalInput")
    si_d = nc.dram_tensor("si", [P, WC], I16, kind="ExternalInput")
    m_d = nc.dram_tensor("m", [P, WM], F32, kind="ExternalInput")
    b2_d = nc.dram_tensor("b2r", [P, CLASSES], F32, kind="ExternalInput")
    out_d = nc.dram_tensor("out2", [BLOCKS_PER_CORE, P, CLASSES], F32,
                           kind="ExternalOutput")

    qctr = [0]

    def qrr():
        qctr[0] = (qctr[0] + 1) % NQ
        return qctr[0]

    with tile.TileContext(nc) as tc:
        with tc.tile_pool(name="sbuf", bufs=2) as pool, \
             tc.tile_pool(name="persist", bufs=1) as pers, \
             tc.tile_pool(name="psum", bufs=2, space="PSUM") as pp:
            mshift = pers.tile([P, 1], F32)
            nc.gpsimd.memset(mshift[:], -M_SHIFT)
            b2t = pers.tile([P, CLASSES], F32)
            nc.sync.dma_start(b2t[:], b2_d[:])

            def body():
                si = pers.tile([P, WC], I16, tag="si_all")
                nc.sync.dma_start(si[:], si_d[:])
                mt = pers.tile([P, WM], F32, tag="m_all")
                nc.sync.dma_start(mt[:], m_d[:])
                woffs, moffs = [], []
                w = 0
                mo = 0
                for j in range(BLOCKS_PER_CORE):
                    woffs.append(w)
                    moffs.append(mo)
                    w += (DsC[j] + 1) * 8
                    mo += DsC[j] + 1
                for (j0, gc, D) in groupsC:
                    De = D - 1
                    X = pool.tile([P, CH, G2UNIT], F16, tag="X")
                    for b in range(gc):
                        Xb = X[:, b * D:(b + 1) * D, :]
                        _gather_calls(nc, Xb, 0, D, g4_d[:, :], si,
                                      woffs[j0 + b], G2UNIT, qrr)

                    parf = mt[:, moffs[j0]:moffs[j0] + gc * D]
                    par = parf.rearrange("p (g c) -> p g c", g=gc)
                    Xlo = _gv3(X, gc, D, 0, D, 0, 18)
                    Xhi = _gv3(X, gc, D, 0, D, G2SUB, G2SUB + 18)
                    esd = pool.tile([P, CH, 18], F16, tag="esd")
                    esdv = _gv3(esd, gc, D, 0, D, 0, 18)
                    nc.vector.tensor_tensor(out=esdv, in0=Xhi, in1=Xlo, op=sub)
                    es = pool.tile([P, CH, 18], F16, tag="es")
                    esv = _gv3(es, gc, D, 0, D, 0, 18)
                    nc.vector.tensor_tensor(
                        out=esv, in0=esdv,
                        in1=par[:, :, :, None].to_broadcast([P, gc, D, 18]),
                        op=mult)
                    nc.vector.tensor_tensor(out=esv, in0=esv, in1=Xlo, op=add)

                    t_t = pool.tile([P, CH, 1], F16, tag="t")
                    tv = _gv3(t_t, gc, De, 0, De, 0, 1)
                    nc.vector.tensor_tensor(
                        out=tv, in0=_gv3(es, gc, D, 1, D, 16, 17),
                        in1=_gv3(es, gc, D, 0, 1, 17, 18).to_broadcast([P, gc, De, 1]),
                        op=add)
                    e1 = pool.tile([P, CH, 1], F16, tag="e1")
                    e1v = _gv3(e1, gc, De, 0, De, 0, 1)
                    nc.scalar.activation(e1v, tv,
                                         mybir.ActivationFunctionType.Exp,
                                         bias=mshift[:])
                    e2 = pool.tile([P, CH, 1], F16, tag="e2")
                    e2v = _gv3(e2, gc, De, 0, De, 0, 1)
                    nc.scalar.activation(e2v, tv,
                                         mybir.ActivationFunctionType.Exp,
                                         bias=mshift[:], scale=NEG)
                    we = pool.tile([P, CH, 1], F16, tag="we")
                    wev = _gv3(we, gc, De, 0, De, 0, 1)
                    nc.vector.tensor_tensor(out=wev, in0=e1v, in1=e2v, op=mx)

                    wx = pool.tile([P, CH, 17], F16, tag="wx")
                    nc.vector.tensor_tensor(
                        out=_gv3(wx, gc, De, 0, De, 0, 16),
                        in0=_gv3(es, gc, D, 1, D, 0, 16),
                        in1=wev.to_broadcast([P, gc, De, 16]),
                        op=mult)
                    nc.scalar.copy(_gv3(wx, gc, De, 0, De, 16, 17), wev)

                    acc = _tree_reduce2(nc, pool, wx, gc, De, De, 17, "trC",
                                        CH // 2 + 4)

                    recip = pool.tile([P, 4, 1], F32, tag="recip")
                    nc.vector.reciprocal(recip[:, 0:gc], acc[:, 0:gc, 16:17])
                    o2 = pool.tile([P, 4, CLASSES], F32, tag="o2")
                    nc.vector.tensor_tensor(
                        out=o2[:, 0:gc], in0=acc[:, 0:gc, 0:16],
                        in1=recip[:, 0:gc].to_broadcast([P, gc, CLASSES]),
                        op=mult)
                    nc.vector.tensor_tensor(
                        out=o2[:, 0:gc], in0=o2[:, 0:gc],
                        in1=b2t[:, None, :].to_broadcast([P, gc, CLASSES]),
                        op=add)
                    for b in range(gc):
                        nc.sync.dma_start(out_d[j0 + b], o2[:, b, :])

            for _ in range(reps):
                body()
    nc.finalize()
    return nc


# ------------------------------ runner plumbing ------------------------------

def make_runner(nc, n_cores=N_CORES):
    """Cached shard_map runner for a finalized Bass module."""
    import jax
    from jax.sharding import Mesh, PartitionSpec
    from jax.experimental.shard_map import shard_map
    from concourse.bass2jax import _bass_exec_p, install_neuronx_cc_hook, partition_id_tensor

    install_neuronx_cc_hook()
    partition_name = nc.partition_id_tensor.name if nc.partition_id_tensor else None
    in_names, out_names, out_avals = [], [], []
    for alloc in nc.m.functions[0].allocations:
        if not isinstance(alloc, mybir.MemoryLocationSet):
            continue
        name = alloc.memorylocations[0].name
        if alloc.kind == "ExternalInput":
            if name != partition_name:
                in_names.append(name)
        elif alloc.kind == "ExternalOutput":
            out_names.append(name)
            out_avals.append(jax.core.ShapedArray(tuple(alloc.tensor_shape),
                                                  mybir.dt.np(alloc.dtype)))
    n_params = len(in_names)
    all_names = in_names + out_names + ([partition_name] if partition_name else [])

    def _body(*args):
        operands = list(args)
        if partition_name is not None:
            operands.append(partition_id_tensor())
        return tuple(_bass_exec_p.bind(
            *operands, out_avals=tuple(out_avals), in_names=tuple(all_names),
            out_names=tuple(out_names), lowering_input_output_aliases=(),
            sim_require_finite=False, sim_require_nnan=False, nc=nc))

    devices = jax.devices()[:n_cores]
    mesh = Mesh(np.asarray(devices), ("core",))
    sharded = jax.jit(
        shard_map(_body, mesh=mesh,
                  in_specs=(PartitionSpec("core"),) * (n_params + len(out_names)),
                  out_specs=(PartitionSpec("core"),) * len(out_names),
                  check_rep=False),
        keep_unused=True)

    import jax as _jax
    from jax.sharding import NamedSharding

    _dev_args = {}

    def run(in_maps, key=None, raw=False):
        if key is not None and key in _dev_args:
            args = _dev_args[key]
        else:
            concat_in = [np.concatenate([np.asarray(m[nm]) for m in in_maps], axis=0)
                         for nm in in_names]
            concat_zero = [np.zeros((n_cores * a.shape[0], *a.shape[1:]), a.dtype)
                           for a in out_avals]
            sh = NamedSharding(mesh, PartitionSpec("core"))
            args = [_jax.device_put(a, sh) for a in concat_in + concat_zero]
            _jax.block_until_ready(args)
            if key is not None:
                _dev_args[key] = args
        outs = sharded(*args)
        _jax.block_until_ready(outs)
        if raw:
            return outs
        return [
            {nm: np.asarray(outs[i]).reshape(n_cores, *out_avals[i].shape)[c]
             for i, nm in enumerate(out_names)}
            for c in range(n_cores)
        ]

    return run


def _get_compiled(key, builder):
    if key not in _cache:
        nc = builder()
        _cache[key] = make_runner(nc)
    return _cache[key]


# --------------------------------- kernel ------------------------------------

def kernel(x, edge_index, W1, a_src1, a_dst1, b1, W2, a_src2, a_dst2, b2):
    x = np.asarray(x, np.float32)
    edge_index = np.asarray(edge_index)
    W1 = np.asarray(W1, np.float32)
    W2 = np.asarray(W2, np.float32)
    a_src1 = np.asarray(a_src1, np.float32)
    a_dst1 = np.asarray(a_dst1, np.float32)
    a_src2 = np.asarray(a_src2, np.float32)
    a_dst2 = np.asarray(a_dst2, np.float32)
    b1 = np.asarray(b1, np.float32)
    b2 = np.asarray(b2, np.float32)

    cfg = host_prep(edge_index)

    As = np.zeros((P, HEADS), np.float32)
    Ad = np.zeros((P, HEADS), np.float32)
    for h in range(HEADS):
        As[h * HID:(h + 1) * HID, h] = a_src1[h]
        Ad[h * HID:(h + 1) * HID, h] = a_dst1[h]
    # feature cols in (k, h) order: stored col i = logical col PERM_KH[i]
    W1p = W1[:, PERM_KH]
    W1ext = np.concatenate([W1p, W1 @ As, W1 @ Ad], 1).astype(np.float32)
    W2p = W2[PERM_KH, :]
    W2ext = np.concatenate([W2p, W2p @ a_src2.T, W2p @ a_dst2.T], 1).astype(np.float32)
    ident = np.eye(P, dtype=np.float32)
    b1p = b1[PERM_KH]
    b1r = np.ascontiguousarray(np.broadcast_to(b1p, (P, P))).astype(np.float32)
    b2r = np.ascontiguousarray(np.broadcast_to(b2, (P, CLASSES))).astype(np.float32)

    xT = np.zeros((P, N_PAD), np.float32)
    xT[:, :N] = x.T

    # ---- NEFF-A ----
    run_a = _get_compiled("A", build_neff_a)
    import ml_dtypes
    xTb = xT.astype(ml_dtypes.bfloat16)
    W1b = W1ext.astype(ml_dtypes.bfloat16)
    in_a = [{"xT": np.ascontiguousarray(xTb[:, k * NODES_PER_CORE:(k + 1) * NODES_PER_CORE]),
             "w1e": W1b} for k in range(N_CORES)]
    res_a = run_a(in_a)
    g_all = np.zeros((N_PAD, 144), np.float16)
    for k in range(N_CORES):
        blk = res_a[k]["g_out"].transpose(1, 0, 2).reshape(-1, 144)
        g_all[k * NODES_PER_CORE:(k + 1) * NODES_PER_CORE] = blk

    # assemble B table: row_of(node) = node except 0 -> 50000; rows 0 and
    # 50175 are poison (alpha_src = -1000 -> padded slots weigh 0)
    g_full = np.zeros((N_PAD, GROW), np.float16)
    g_full[1:N, 0:144] = g_all[1:N]
    g_full[NODE0_ROW, 0:144] = g_all[0]
    g_full[POISON_LO, 128:136] = -80.0
    g_full[POISON_HI, 128:136] = -80.0

    # ---- NEFF-B ----
    keyb = ("B", tuple(cfg["LsB"]), tuple(cfg["HsB"]))
    run_b = _get_compiled(keyb, lambda: build_neff_b(cfg))
    w2cs = -W2ext.sum(axis=0, keepdims=True).astype(np.float16)
    ones1 = np.ones((1, P), np.float16)
    in_b = [{"g": g_full, "si": cfg["siB"][k], "m": cfg["mB"][k],
             "w2e": W2ext.astype(np.float16), "w2cs": w2cs, "ones1": ones1,
             "b1r": b1r.astype(np.float16), "ident": ident.astype(np.float16)}
            for k in range(N_CORES)]
    res_b = run_b(in_b)
    g2_full = np.zeros((N_PAD, 18), np.float32)
    for k in range(N_CORES):
        rows = cfg["binsB"].reshape(NBINS, P)[np.arange(BLOCKS_PER_CORE) * 8 + k]
        g2_full[rows.reshape(-1)] = res_b[k]["g2_out"].reshape(-1, 18).astype(np.float32)
    g2_full[~np.isfinite(g2_full).all(1)] = 0
    g2_full[N:] = 0

    # pair-packed table for C; poison unit for padded slots
    g4 = np.zeros((NUNIT, G2UNIT), np.float16)
    g4[:, 0:18] = g2_full[0::2]
    g4[:, G2SUB:G2SUB + 18] = g2_full[1::2]
    g4[POISON_UNIT, 16] = -80.0
    g4[POISON_UNIT, G2SUB + 16] = -80.0

    # ---- NEFF-C ----
    keyc = ("C", tuple(cfg["DsC"]))
    run_c = _get_compiled(keyc, lambda: build_neff_c(cfg))
    in_c = [{"g4": g4, "si": cfg["siC"][k], "m": cfg["mC"][k],
             "b2r": b2r} for k in range(N_CORES)]
    res_c = run_c(in_c)

    out = np.zeros((N_PAD, CLASSES), np.float32)
    for k in range(N_CORES):
        rows = cfg["binsC"].reshape(NBINS, P)[np.arange(BLOCKS_PER_CORE) * 8 + k]
        out[rows.reshape(-1)] = res_c[k]["out2"].reshape(-1, CLASSES)

    global _last_cfg, _last_inputs
    _last_cfg = cfg
    _last_inputs = {"A": in_a, "B": in_b, "C": in_c}
    return out[:N].astype(np.float32)
